# revision 52
# baseline (speedup 1.0000x reference)
"""BlockTransformerPairBias Trainium2 kernel (v2: phase-overlapped).

Sharding: 8 cores = (batch 0/1) x (4 groups of 16 attention blocks).
Each core computes its 1024 tokens end-to-end; no collectives.

v2 layout: one interleaved emission pass so every engine stays dense —
the cond-table phase (vector/scalar heavy), the pair-bias matmuls
(tensor heavy) and the input LN run woven together in one u-loop;
attention runs as a pipelined loop with bias tiles prefetched; the
transition overlaps the attention tail.  The bias reshape round-trips
DRAM in bf16.  PSUM is split 2/2/4 banks across transpose / matmul /
wide tags so consecutive iterations double-buffer.
"""

import sys

sys.path.insert(0, "/opt/trn_rl_repo")

from contextlib import ExitStack

import numpy as np
import ml_dtypes

import concourse.bass as bass
import concourse.tile as tile
from concourse import bacc, mybir
from concourse.bass_utils import run_bass_kernel_spmd
from concourse.masks import make_identity

F32 = mybir.dt.float32
BF16 = mybir.dt.bfloat16
F8 = mybir.dt.float8e4
I16 = mybir.dt.int16
AF = mybir.ActivationFunctionType
ALU = mybir.AluOpType
BF = ml_dtypes.bfloat16
F8NP = ml_dtypes.float8_e4m3

B, N, NRES = 2, 4096, 1024
CS, CC, CZ, H, BLK = 512, 384, 128, 8, 64
CH = CS // H          # 64
NB = N // BLK         # 64
NCORES = 8
NBLK = NB * B // NCORES   # 16 blocks per core
NT = NBLK * BLK           # 1024 tokens per core
RT = NT // 128            # 8 token tiles
EPS = 1e-5

_CACHE = {}


def _declare(nc):
    t = {}

    def inp(name, shape, dt):
        t[name] = nc.dram_tensor(name, list(shape), dt, kind="ExternalInput").ap()

    inp("re", (NT, CS), BF16)
    inp("zz", (NBLK, CZ, 2, BLK * BLK), F8)
    inp("s", (NRES, CC), BF16)
    inp("idx", (128, NT // 16), I16)
    inp("wq", (128, 4, CS), BF16)
    inp("wk", (128, 4, CS), BF16)
    inp("wv", (128, 4, CS), BF16)
    inp("wg", (128, 4, CS), BF16)
    inp("wout", (128, 4, CS), BF16)
    inp("w1", (128, 4, 2 * CS), BF16)
    inp("w2", (128, 4, 2 * CS), BF16)
    inp("wb", (128, 8, CS), BF16)
    inp("wada", (128, 3, 3 * CS), BF16)
    inp("wbs", (CZ, 64), F8)
    inp("svec", (H,), F32)          # holds MINUS S[h]
    inp("bq", (128, 4), F32)
    inp("bk", (128, 4), F32)
    inp("bada", (3 * CS,), BF16)
    t["out"] = nc.dram_tensor("out", [NT, CS], F32, kind="ExternalOutput").ap()
    return t


def _bcast(ap, p=128):
    """Broadcast a 1-D DRAM AP across p partitions."""
    return bass.AP(tensor=ap.tensor, offset=ap.offset, ap=[[0, p]] + list(ap.ap))


def _b0(ap_, reps, at=None):
    """Insert a broadcast dim of length `reps` into an AP."""
    lst = list(ap_.ap)
    pos = len(lst) if at is None else at
    lst.insert(pos, [0, reps])
    return bass.AP(tensor=ap_.tensor, offset=ap_.offset, ap=lst)


def _emit(ctx, tc, t, flags):
    nc = tc.nc
    has_bq, has_bk, has_bag, has_bab, has_btg = flags

    consts = ctx.enter_context(tc.tile_pool(name="consts", bufs=1))
    acts = ctx.enter_context(tc.tile_pool(name="acts", bufs=1))
    sb = ctx.enter_context(tc.tile_pool(name="sb", bufs=2))
    ps_tp = ctx.enter_context(tc.tile_pool(name="ps_tp", bufs=2, space="PSUM"))
    ps_mm = ctx.enter_context(tc.tile_pool(name="ps_mm", bufs=2, space="PSUM"))
    ps_pz = ctx.enter_context(tc.tile_pool(name="ps_pz", bufs=4, space="PSUM"))
    dramp = ctx.enter_context(tc.tile_pool(name="dram", bufs=1, space="DRAM"))
    dpp = ctx.enter_context(tc.tile_pool(name="dramP", bufs=16, space="DRAM"))

    # ---- constants / weights resident all kernel ----
    ident = consts.tile([128, 128], BF16)
    make_identity(nc, ident[:])
    eps_t = consts.tile([128, 1], F32)
    nc.vector.memset(eps_t[:], EPS)
    wbs_sb = consts.tile([CZ, 64], F8)
    svec_sb = consts.tile([128, H], F32)
    idx_sb = consts.tile([128, NT // 16], I16)
    bq_sb = bk_sb = None
    if has_bq:
        bq_sb = consts.tile([128, 4], F32)
        nc.sync.dma_start(bq_sb[:], t["bq"][:])
    if has_bk:
        bk_sb = consts.tile([128, 4], F32)
        nc.sync.dma_start(bk_sb[:], t["bk"][:])
    wq = consts.tile([128, 4, CS], BF16)
    wk = consts.tile([128, 4, CS], BF16)
    wv = consts.tile([128, 4, CS], BF16)
    wg = consts.tile([128, 4, CS], BF16)
    wout = consts.tile([128, 4, CS], BF16)

    # ---- persistent activations ----
    re = acts.tile([128, RT, CS], BF16)
    _re_src = t["re"].rearrange("(r p) c -> p r c", p=128)
    nc.sync.dma_start(wbs_sb[:], t["wbs"][:])
    nc.sync.dma_start(svec_sb[:], _bcast(t["svec"]))
    nc.sync.dma_start(idx_sb[:], t["idx"][:])
    for _r in range(RT):
        nc.sync.dma_start(re[:, _r, :], _re_src[:, _r, :])
    h_sb = acts.tile([128, RT, CS], BF16)
    xnT = acts.tile([128, 4, NT], BF16)
    qf = acts.tile([128, 4, NT], BF16)
    kf = acts.tile([128, 4, NT], BF16)
    qf2 = acts.tile([64, 4, NT], BF16)
    kf2 = acts.tile([64, 4, NT], BF16)
    gsig = acts.tile([128, RT, CS], BF16)
    bias_aa = acts.tile([128, RT, H, 64], BF16)
    w1 = acts.tile([128, 4, 2 * CS], BF16)
    w2 = acts.tile([128, 4, 2 * CS], BF16)
    wb = acts.tile([128, 8, CS], BF16)

    tbl = dramp.tile([NRES, 3 * CS], BF16)

    def ln_rstd(x_ap):
        """bn stats over free dim -> (mean [P,1], rstd [P,1]) tiles."""
        p = x_ap.shape[0]
        stats = sb.tile([128, 6], F32, tag="stats")
        nc.vector.bn_stats(stats[:p], x_ap)
        mv = sb.tile([128, 2], F32, tag="mv")
        nc.vector.bn_aggr(mv[:p], stats[:p])
        sd = sb.tile([128, 1], F32, tag="sd")
        nc.scalar.activation(sd[:p], mv[:p, 1:2], AF.Sqrt, bias=eps_t[:p], scale=1.0)
        nc.vector.reciprocal(sd[:p], sd[:p])
        return mv[:p, 0:1], sd[:p]

    from concourse.tile import add_dep_helper

    def attention(gp):
        # ---- v for both blocks first (mm slots free early) ----
        vts = []
        for g2 in range(2):
            g = 2 * gp + g2
            vp = ps_mm.tile([128, CS], F32, tag="mm")
            for k in range(4):
                nc.tensor.matmul(vp[0:64, :], xnT[:, k, g * 64:(g + 1) * 64],
                                 wv[:, k, :], start=(k == 0), stop=(k == 3))
            vt = sb.tile([64, CS], BF16, tag="vt", bufs=4)
            nc.vector.tensor_copy(vt[:], vp[0:64, :])
            vts.append(vt)
        # ---- attention: all heads, both blocks ----
        sc_ps = ps_pz.tile([128, CS], F32, tag="pz")
        for g2 in range(2):
            g = 2 * gp + g2
            for h in range(H):
                m = h // 2
                qsl = (qf[0:64, m, g * 64:(g + 1) * 64] if h % 2 == 0
                       else qf2[:, m, g * 64:(g + 1) * 64])
                ksl = (kf[0:64, m, g * 64:(g + 1) * 64] if h % 2 == 0
                       else kf2[:, m, g * 64:(g + 1) * 64])
                nc.tensor.matmul(sc_ps[g2 * 64:g2 * 64 + 64,
                                       h * 64:(h + 1) * 64],
                                 qsl, ksl, start=True, stop=True,
                                 tile_position=(0, g2 * 64))
        nc.vector.tensor_add(sc_ps[:].rearrange("p (h j) -> p h j", h=H),
                             sc_ps[:].rearrange("p (h j) -> p h j", h=H),
                             bias_aa[:, gp, :, :])
        a_sb = sb.tile([128, CS], BF16, tag="a_sb", bufs=3)
        nc.scalar.activation(a_sb[:], sc_ps[:], AF.Exp)
        rs = sb.tile([128, H], F32, tag="rs")
        nc.vector.tensor_reduce(rs[:], a_sb[:].rearrange(
            "p (h j) -> p h j", h=H), axis=mybir.AxisListType.X, op=ALU.add)
        rcp = sb.tile([128, H], F32, tag="rcp")
        nc.vector.reciprocal(rcp[:], rs[:])
        # fold softmax normalizer into the sigmoid gate
        gg = sb.tile([128, H, 64], BF16, tag="gg")
        nc.vector.tensor_mul(
            gg[:], gsig[:, gp, :].rearrange("p (h j) -> p h j", h=H),
            _b0(rcp[:], 64))

        o_ps = ps_pz.tile([128, CS], F32, tag="pz")
        for g2 in range(2):
            g = 2 * gp + g2
            vt = vts[g2]
            idq = ident[g2 * 64:g2 * 64 + 64, g2 * 64:g2 * 64 + 64]
            aT_ps = ps_tp.tile([64, CS], BF16, tag="tp")
            for h in range(H):
                nc.tensor.transpose(aT_ps[:, h * 64:(h + 1) * 64],
                                    a_sb[g2 * 64:g2 * 64 + 64,
                                         h * 64:(h + 1) * 64], idq)
            aT_sb = sb.tile([64, CS], BF16, tag="aT_sb", bufs=3)
            nc.vector.tensor_copy(aT_sb[:], aT_ps[:])
            for h in range(H):
                nc.tensor.matmul(
                    o_ps[g2 * 64:g2 * 64 + 64, h * 64:(h + 1) * 64],
                    aT_sb[:, h * 64:(h + 1) * 64],
                    vt[:, h * 64:(h + 1) * 64],
                    start=True, stop=True, tile_position=(0, g2 * 64))
        og_pair = sb.tile([128, CS], BF16, tag="og_pair", bufs=3)
        nc.vector.tensor_mul(og_pair[:].rearrange("p (h j) -> p h j", h=H),
                             o_ps[:].rearrange("p (h j) -> p h j", h=H),
                             gg[:])
        ogT = sb.tile([128, 4, 128], BF16, tag="ogT")
        for c in range(4):
            tp = ps_tp.tile([128, 128], BF16, tag="tp")
            nc.tensor.transpose(tp[:], og_pair[:, c * 128:(c + 1) * 128],
                                ident[:])
            nc.vector.tensor_copy(ogT[:, c, :], tp[:])
        # ---- Wout + residual (tp tag: freed late, off the mm path) ----
        pt = ps_tp.tile([128, CS], F32, tag="tp")
        for k in range(4):
            nc.tensor.matmul(pt[:], ogT[:, k, :], wout[:, k, :],
                             start=(k == 0), stop=(k == 3))
        nc.vector.tensor_add(h_sb[:, gp, :], pt[:], re[:, gp, :])

    # =============== phase A: LN1 + cond tables + bias path ===============
    with tc.tile_pool(name="pa", bufs=1) as pa, \
         tc.tile_pool(name="paw", bufs=2) as paw:
        wada = pa.tile([128, 3, 3 * CS], BF16)
        nc.scalar.dma_start(wada[:], t["wada"][:])
        bada_bc = pa.tile([128, 3 * CS], BF16)
        if has_bag or has_bab or has_btg:
            nc.sync.dma_start(bada_bc[:], _bcast(t["bada"]))

        dPs = {}

        def bias_block(g):
            """Pair-bias matmuls for block g -> dP(bf16) -> Pr_all[gp] half."""
            gp, g2 = g // 2, g % 2
            zt = paw.tile([CZ, 2, BLK * BLK], F8, tag="zt")
            zq = (nc.gpsimd, nc.scalar)[g % 2]
            zq.dma_start(zt[:], t["zz"][g])
            ze = ps_pz.tile([128, 512], F32, tag="pz")
            zo = ps_pz.tile([128, 512], F32, tag="pz")
            # z pass writes P rows 0..8 of each 32-strip; z^2 pass (host
            # precomputed) accumulates E[z^2] into row 9.  Each strip's
            # accumulation group closes before the next opens.
            for cg in range(4):
                tpos = (0, 32 * cg)
                rows = slice(32 * cg, 32 * cg + 32)
                ev = slice((2 * cg) * 512, (2 * cg + 1) * 512)
                od = slice((2 * cg + 1) * 512, (2 * cg + 2) * 512)
                nc.tensor.matmul(ze[rows, :], wbs_sb[:, 0:32], zt[:, 0, ev],
                                 start=True, stop=False, tile_position=tpos)
                nc.tensor.matmul(zo[rows, :], wbs_sb[:, 0:32], zt[:, 0, od],
                                 start=True, stop=False, tile_position=tpos)
                nc.tensor.matmul(ze[rows, :], wbs_sb[:, 32:64], zt[:, 1, ev],
                                 start=False, stop=True, tile_position=tpos)
                nc.tensor.matmul(zo[rows, :], wbs_sb[:, 32:64], zt[:, 1, od],
                                 start=False, stop=True, tile_position=tpos)
            Psbb = sb.tile([128, 1024], BF16, tag="Psbb")
            nc.vector.tensor_copy(Psbb[:, 0:512], ze[:])
            nc.vector.tensor_copy(Psbb[:, 512:1024], zo[:])
            # round-trip through DRAM to reshape [32cg+m, (ab i3 j)]
            # -> [i=(cg ab i3), m, j]; the strided re-load happens in
            # phase B so only dP (DRAM) holds the 16 blocks.
            dP = dpp.tile([128, 1024], BF16, tag="dP")
            st = nc.gpsimd.dma_start(dP[:], Psbb[:])
            dPs[g] = (dP, st.ins)

        def p1_tile(r):
            """Cond-table tile r: LN(s) @ [W_ada_gate|W_ada_bias|W_tgate]."""
            s_t = paw.tile([128, CC], BF16, tag="s_t")
            nc.sync.dma_start(s_t[:], t["s"][r * 128:(r + 1) * 128, :])
            mean, rstd = ln_rstd(s_t[:])
            cond = sb.tile([128, CC], BF16, tag="cond")
            nc.vector.tensor_scalar(out=cond[:], in0=s_t[:], scalar1=mean,
                                    scalar2=rstd, op0=ALU.subtract, op1=ALU.mult)
            ct = sb.tile([128, 3, 128], BF16, tag="ct")
            for c in range(3):
                tp = ps_tp.tile([128, 128], BF16, tag="tp")
                nc.tensor.transpose(tp[:], cond[:, c * 128:(c + 1) * 128], ident[:])
                nc.scalar.copy(ct[:, c, :], tp[:])
            tbl_sb = sb.tile([128, 3 * CS], BF16, tag="tbl_sb")
            for n in range(3):
                pt = ps_mm.tile([128, CS], F32, tag="mm")
                for k in range(3):
                    nc.tensor.matmul(pt[:], ct[:, k, :],
                                     wada[:, k, n * CS:(n + 1) * CS],
                                     start=(k == 0), stop=(k == 2))
                seg = slice(n * CS, (n + 1) * CS)
                if n == 0:
                    if has_bag:
                        nc.vector.tensor_add(pt[:], pt[:], bada_bc[:, seg])
                    nc.scalar.activation(tbl_sb[:, seg], pt[:], AF.Sigmoid)
                elif n == 1:
                    if has_bab:
                        nc.vector.tensor_add(tbl_sb[:, seg], pt[:], bada_bc[:, seg])
                    else:
                        nc.scalar.copy(tbl_sb[:, seg], pt[:])
                else:
                    if has_btg:
                        nc.vector.tensor_add(pt[:], pt[:], bada_bc[:, seg])
                    nc.scalar.activation(tbl_sb[:, seg], pt[:], AF.Sigmoid)
            nc.sync.dma_start(tbl[r * 128:(r + 1) * 128, :], tbl_sb[:])

        def ln1_tile(r):
            mean, rstd = ln_rstd(re[:, r, :])
            xn = sb.tile([128, CS], BF16, tag="xn")
            nc.vector.tensor_scalar(out=xn[:], in0=re[:, r, :], scalar1=mean,
                                    scalar2=rstd, op0=ALU.subtract, op1=ALU.mult)
            for c in range(4):
                tp = ps_tp.tile([128, 128], BF16, tag="tp")
                nc.tensor.transpose(tp[:], xn[:, c * 128:(c + 1) * 128], ident[:])
                nc.scalar.copy(xnT[:, c, r * 128:(r + 1) * 128], tp[:])

        def qk_proj(n):
            for (w, bias_sb, has_b, dst) in ((wq, bq_sb, has_bq, qf),
                                             (wk, bk_sb, has_bk, kf)):
                for m in range(4):
                    pt = ps_mm.tile([128, CS], F32, tag="mm")
                    for k in range(4):
                        nc.tensor.matmul(pt[:], w[:, k, m * 128:(m + 1) * 128],
                                         xnT[:, k, n * 512:(n + 1) * 512],
                                         start=(k == 0), stop=(k == 3))
                    dseg = dst[:, m, n * 512:(n + 1) * 512]
                    if has_b:
                        nc.vector.tensor_scalar_add(out=dseg, in0=pt[:],
                                                    scalar1=bias_sb[:, m:m + 1])
                    else:
                        nc.vector.tensor_copy(dseg, pt[:])
            # odd heads' q/k rows duplicated at partition base 0: every QK
            # matmul then issues from PE row-group 0 (mixed row-groups
            # draining into one PSUM bank concurrently crash the device)
            nsl = slice(n * 512, (n + 1) * 512)
            nc.sync.dma_start(qf2[:, :, nsl], qf[64:128, :, nsl])
            nc.sync.dma_start(kf2[:, :, nsl], kf[64:128, :, nsl])

        def g_proj(r):
            pt = ps_mm.tile([128, CS], F32, tag="mm")
            for k in range(4):
                nc.tensor.matmul(pt[:], xnT[:, k, r * 128:(r + 1) * 128],
                                 wg[:, k, :], start=(k == 0), stop=(k == 3))
            nc.scalar.activation(gsig[:, r, :], pt[:], AF.Sigmoid)

        def load_pr(gp):
            Pr = acts.tile([128, 10, 64], BF16, tag="Pr", bufs=2)
            for g2 in range(2):
                dP, st_ins = dPs[2 * gp + g2]
                base = dP[:]
                for cg in range(4):
                    srcap = bass.AP(tensor=base.tensor,
                                    offset=base.offset + cg * 32768,
                                    ap=[[64, 16], [1024, 10], [1, 64]])
                    q = (nc.sync, nc.gpsimd, nc.scalar)[(g2 * 4 + cg) % 3]
                    ld = q.dma_start(
                        Pr[g2 * 64 + cg * 16:g2 * 64 + (cg + 1) * 16, :, :],
                        srcap)
                    add_dep_helper(ld.ins, st_ins, reason="reshape RAW")
            return Pr

        def stats_prep(gp):
            """Pair-bias LN stats + full bias tile, off the critical chain."""
            Pr = load_pr(gp)
            msq = sb.tile([128, 64], F32, tag="msq")
            nc.vector.tensor_mul(msq[:], Pr[:, 8, :], Pr[:, 8, :])
            var_t = sb.tile([128, 64], F32, tag="var_t")
            nc.vector.tensor_sub(var_t[:], Pr[:, 9, :], msq[:])
            nc.scalar.activation(var_t[:], var_t[:], AF.Sqrt,
                                 bias=eps_t[:], scale=1.0)
            rstd_t = sb.tile([128, 64], F32, tag="rstd_t")
            nc.vector.reciprocal(rstd_t[:], var_t[:])
            mr_t = sb.tile([128, 64], F32, tag="mr_t")
            nc.vector.tensor_mul(mr_t[:], Pr[:, 8, :], rstd_t[:])
            # bias[p,(h,j)] = Pr_h*rstd - S_h*mean*rstd  (svec = -S)
            mrs = sb.tile([128, H, 64], BF16, tag="mrs")
            nc.vector.tensor_mul(mrs[:], _b0(mr_t[:], H, at=1),
                                 _b0(svec_sb[:], 64))
            ba = bias_aa[:, gp, :, :]
            nc.vector.tensor_mul(ba, Pr[:, 0:H, :], _b0(rstd_t[:], H, at=1))
            nc.vector.tensor_add(ba, ba, mrs[:])

        for u in range(RT):
            if u >= 1:
                stats_prep(u - 1)
            if u == 1:
                nc.sync.dma_start(wq[:], t["wq"][:])
                nc.sync.dma_start(wk[:], t["wk"][:])
                nc.sync.dma_start(wg[:], t["wg"][:])
            elif u == 2:
                nc.sync.dma_start(wv[:], t["wv"][:])
                nc.sync.dma_start(wout[:], t["wout"][:])
            elif u == 6:
                nc.gpsimd.dma_start(w1[:], t["w1"][:])
                nc.gpsimd.dma_start(w2[:], t["w2"][:])
                nc.gpsimd.dma_start(wb[:], t["wb"][:])
            ln1_tile(u)
            bias_block(2 * u)
            p1_tile(u)
            bias_block(2 * u + 1)
            if u >= 4:
                # fuse: attention for the first half overlaps the back half
                # of phase A (fills PE gaps, keeps HAM warm)
                attention(u - 4)
            if u == 3 or u == 7:
                n = u // 4
                qk_proj(n)
                for r in range(n * 4, n * 4 + 4):
                    g_proj(r)

        stats_prep(RT - 1)

    # =============== phase B/C: attention + transition ===============
    with tc.tile_pool(name="pb", bufs=1) as pb:
        tT = pb.tile([128, 4, NT], BF16)
        bb = pb.tile([128, 8, NT], BF16)
        tgate = pb.tile([128, RT, CS], BF16)

        def gather_r(r):
            gth_t = sb.tile([128, 1, 2 * CS], BF16, tag="gth")
            nc.gpsimd.dma_gather(
                out_ap=gth_t[:], in_ap=tbl[:, 0:2 * CS],
                idxs_ap=idx_sb[:, r * 8:(r + 1) * 8],
                num_idxs=128, num_idxs_reg=128, elem_size=2 * CS,
                elem_step=3 * CS)
            nc.gpsimd.dma_gather(
                out_ap=tgate[:, r:r + 1, :], in_ap=tbl[:, 2 * CS:3 * CS],
                idxs_ap=idx_sb[:, r * 8:(r + 1) * 8],
                num_idxs=128, num_idxs_reg=128, elem_size=CS,
                elem_step=3 * CS)
            return gth_t

        gths = {0: gather_r(0), 1: gather_r(1)}

        def ln2_tile(r, gth_t):
            mean, rstd = ln_rstd(h_sb[:, r, :])
            # t2 = LN(h)*sig(gate) + bias, fused as two scalar_tensor_tensor
            t1 = sb.tile([128, CS], BF16, tag="t1")
            nc.vector.scalar_tensor_tensor(
                out=t1[:], in0=h_sb[:, r, :], scalar=mean,
                in1=gth_t[:, 0, 0:CS], op0=ALU.subtract, op1=ALU.mult)
            t2 = sb.tile([128, CS], BF16, tag="t2")
            nc.vector.scalar_tensor_tensor(
                out=t2[:], in0=t1[:], scalar=rstd,
                in1=gth_t[:, 0, CS:2 * CS], op0=ALU.mult, op1=ALU.add)
            for c in range(4):
                tp = ps_tp.tile([128, 128], BF16, tag="tp")
                nc.tensor.transpose(tp[:], t2[:, c * 128:(c + 1) * 128], ident[:])
                nc.vector.tensor_copy(tT[:, c, r * 128:(r + 1) * 128], tp[:])

        def wb_r(r):
            pt = ps_mm.tile([128, CS], F32, tag="mm")
            for k in range(8):
                nc.tensor.matmul(pt[:], bb[:, k, r * 128:(r + 1) * 128],
                                 wb[:, k, :], start=(k == 0), stop=(k == 7))
            tr = sb.tile([128, CS], F32, tag="tr")
            nc.vector.tensor_mul(tr[:], pt[:], tgate[:, r, :])
            out_t = sb.tile([128, CS], F32, tag="out_t")
            nc.vector.tensor_add(out_t[:], tr[:], h_sb[:, r, :])
            nc.sync.dma_start(t["out"][r * 128:(r + 1) * 128, :], out_t[:])

        def w12(n):
            for m in range(8):
                p1 = ps_mm.tile([128, CS], F32, tag="mm")
                for k in range(4):
                    nc.tensor.matmul(p1[:], w1[:, k, m * 128:(m + 1) * 128],
                                     tT[:, k, n * 512:(n + 1) * 512],
                                     start=(k == 0), stop=(k == 3))
                u1s = sb.tile([128, 512], BF16, tag="u1s")
                nc.scalar.activation(u1s[:], p1[:], AF.Sigmoid)
                u1 = sb.tile([128, 512], BF16, tag="u1")
                nc.vector.tensor_mul(u1[:], u1s[:], p1[:])
                p2 = ps_pz.tile([128, CS], F32, tag="pz")
                for k in range(4):
                    nc.tensor.matmul(p2[:], w2[:, k, m * 128:(m + 1) * 128],
                                     tT[:, k, n * 512:(n + 1) * 512],
                                     start=(k == 0), stop=(k == 3))
                nc.vector.tensor_mul(bb[:, m, n * 512:(n + 1) * 512],
                                     u1[:], p2[:])

        for i in range(4):
            ln2_tile(i, gths.pop(i))
            gths[i + 2] = gather_r(i + 2)
            attention(4 + i)
        w12(0)
        for i in range(4, RT):
            ln2_tile(i, gths.pop(i))
            if i + 2 < RT:
                gths[i + 2] = gather_r(i + 2)
            wb_r(i - 4)
        w12(1)
        for r in range(4, RT):
            wb_r(r)


def build(flags):
    key = ("v2", flags)
    if key in _CACHE:
        return _CACHE[key]
    nc = bacc.Bacc("TRN2", target_bir_lowering=False, debug=False)
    t = _declare(nc)
    with tile.TileContext(nc) as tc:
        with ExitStack() as ctx:
            _emit(ctx, tc, t, flags)
    nc.compile()
    _CACHE[key] = nc
    return nc


def prep_core_inputs(inputs, core):
    """Host-side slicing + weight folding for one core."""
    b = core // 4
    g0 = (core % 4) * NBLK
    r0 = g0 * BLK

    f = lambda k: np.asarray(inputs[k], np.float32)
    ln_w, ln_b = f("ln_w"), f("ln_b")
    sc = 1.0 / np.sqrt(CH)

    def fold(w, scale=1.0):
        return ln_w[:, None] * np.asarray(w, np.float32) * scale

    def foldb(w, scale=1.0):
        return (ln_b @ np.asarray(w, np.float32)) * scale

    Wkv = f("Wkv")
    wq_h, bq_h = fold(inputs["Wq"], sc), foldb(inputs["Wq"], sc)
    wk_h, bk_h = fold(Wkv[:, :CS]), foldb(Wkv[:, :CS])
    wv_h, bv_h = fold(Wkv[:, CS:]), foldb(Wkv[:, CS:])
    wg_h, bg_h = fold(inputs["Wgate"]), foldb(inputs["Wgate"])
    if np.any(bv_h) or np.any(bg_h):
        raise NotImplementedError("nonzero folded v/gate bias unsupported")

    cw = f("adaln_cond_w")
    wada_h = np.concatenate(
        [cw[:, None] * f("W_ada_gate"), cw[:, None] * f("W_ada_bias"),
         cw[:, None] * f("W_tgate")], axis=1)
    bada_h = np.concatenate(
        [f("b_ada_gate"), np.zeros(CS, np.float32), f("b_tgate")]).astype(BF)

    wbias = f("bias_ln_w")[:, None] * f("Wbias")      # [128, 8]
    svec_h = (-wbias.sum(0)).astype(np.float32)       # minus S
    wbs_h = np.zeros((CZ, 64), np.float32)
    wbs_h[:, :H] = wbias
    wbs_h[:, 8] = 1.0 / CZ       # sum column directly produces the mean
    wbs_h[:, 32 + 9] = 1.0 / CZ  # z^2 pass accumulates E[z^2] into row 9

    def ktile(w, kt):
        w = np.asarray(w, np.float32)
        return np.ascontiguousarray(
            w.reshape(kt, 128, w.shape[1]).transpose(1, 0, 2)).astype(BF)

    # framepair: [16, 64, 64, 128] -> [16, 128, 4096] bf16
    fp = np.asarray(inputs["framepair_embed"][b, g0:g0 + NBLK], np.float32)
    zT = np.ascontiguousarray(
        fp.reshape(NBLK, BLK * BLK, CZ).transpose(0, 2, 1))
    zzs = np.stack([zT, zT * zT], axis=2).astype(F8NP)   # [NBLK, CZ, 2, 4096]

    idx = np.asarray(inputs["rigids_to_res_idx"][b, r0:r0 + NT]).astype(np.int16)
    idx_w = np.empty((128, NT // 16), np.int16)
    for p in range(16):
        idx_w[p] = idx[p::16]
    idx_w[16:] = np.tile(idx_w[:16], (7, 1))

    re_f32 = np.ascontiguousarray(inputs["rigids_embed"][b, r0:r0 + NT]).astype(np.float32)
    return {
        "re": re_f32.astype(BF),
        "zz": zzs,
        "s": np.ascontiguousarray(inputs["s"][b]).astype(BF),
        "idx": idx_w,
        "wq": ktile(wq_h, 4), "wk": ktile(wk_h, 4), "wv": ktile(wv_h, 4),
        "wg": ktile(wg_h, 4), "wout": ktile(inputs["Wout"], 4),
        "w1": ktile(inputs["W1"], 4), "w2": ktile(inputs["W2"], 4),
        "wb": ktile(inputs["Wb"], 8), "wada": ktile(wada_h, 3),
        "wbs": wbs_h.astype(F8NP), "svec": svec_h,
        "bq": np.ascontiguousarray(bq_h.reshape(4, 128).T),
        "bk": np.ascontiguousarray(bk_h.reshape(4, 128).T),
        "bada": bada_h,
    }, (bool(np.any(bq_h)), bool(np.any(bk_h)), bool(np.any(f("b_ada_gate"))),
        False, bool(np.any(f("b_tgate"))))


def kernel(**inputs):
    mask = np.asarray(inputs["rigids_mask"])
    if not np.all(mask == 1.0):
        print("WARNING: rigids_mask not all ones; kernel assumes ones", file=sys.stderr)

    in_maps, flags = [], None
    for core in range(NCORES):
        m, flags = prep_core_inputs(inputs, core)
        in_maps.append(m)

    nc = build(flags)
    res = run_bass_kernel_spmd(nc, in_maps, core_ids=list(range(NCORES)))

    out = np.empty((B, N, CS), np.float32)
    for core in range(NCORES):
        b = core // 4
        r0 = (core % 4) * NT
        out[b, r0:r0 + NT] = res.results[core]["out"]
    return out


# revision 53
# speedup vs baseline: 1.0035x; 1.0035x over previous
"""BlockTransformerPairBias Trainium2 kernel (v2: phase-overlapped).

Sharding: 8 cores = (batch 0/1) x (4 groups of 16 attention blocks).
Each core computes its 1024 tokens end-to-end; no collectives.

v2 layout: one interleaved emission pass so every engine stays dense —
the cond-table phase (vector/scalar heavy), the pair-bias matmuls
(tensor heavy) and the input LN run woven together in one u-loop;
attention runs as a pipelined loop with bias tiles prefetched; the
transition overlaps the attention tail.  The bias reshape round-trips
DRAM in bf16.  PSUM is split 2/2/4 banks across transpose / matmul /
wide tags so consecutive iterations double-buffer.
"""

import sys

sys.path.insert(0, "/opt/trn_rl_repo")

from contextlib import ExitStack

import numpy as np
import ml_dtypes

import concourse.bass as bass
import concourse.tile as tile
from concourse import bacc, mybir
from concourse.bass_utils import run_bass_kernel_spmd
from concourse.masks import make_identity

F32 = mybir.dt.float32
BF16 = mybir.dt.bfloat16
F8 = mybir.dt.float8e4
I16 = mybir.dt.int16
AF = mybir.ActivationFunctionType
ALU = mybir.AluOpType
BF = ml_dtypes.bfloat16
F8NP = ml_dtypes.float8_e4m3

B, N, NRES = 2, 4096, 1024
CS, CC, CZ, H, BLK = 512, 384, 128, 8, 64
CH = CS // H          # 64
NB = N // BLK         # 64
NCORES = 8
NBLK = NB * B // NCORES   # 16 blocks per core
NT = NBLK * BLK           # 1024 tokens per core
RT = NT // 128            # 8 token tiles
EPS = 1e-5

_CACHE = {}


def _declare(nc):
    t = {}

    def inp(name, shape, dt):
        t[name] = nc.dram_tensor(name, list(shape), dt, kind="ExternalInput").ap()

    inp("re", (NT, CS), BF16)
    inp("zz", (NBLK, CZ, 2, BLK * BLK), F8)
    inp("s", (NRES, CC), BF16)
    inp("idx", (128, NT // 16), I16)
    inp("wq", (128, 4, CS), BF16)
    inp("wk", (128, 4, CS), BF16)
    inp("wv", (128, 4, CS), BF16)
    inp("wg", (128, 4, CS), BF16)
    inp("wout", (128, 4, CS), BF16)
    inp("w1", (128, 4, 2 * CS), BF16)
    inp("w2", (128, 4, 2 * CS), BF16)
    inp("wb", (128, 8, CS), BF16)
    inp("wada", (128, 3, 3 * CS), BF16)
    inp("wbs", (CZ, 64), F8)
    inp("svec", (H,), F32)          # holds MINUS S[h]
    inp("bq", (128, 4), F32)
    inp("bk", (128, 4), F32)
    inp("bada", (3 * CS,), BF16)
    t["out"] = nc.dram_tensor("out", [NT, CS], F32, kind="ExternalOutput").ap()
    return t


def _bcast(ap, p=128):
    """Broadcast a 1-D DRAM AP across p partitions."""
    return bass.AP(tensor=ap.tensor, offset=ap.offset, ap=[[0, p]] + list(ap.ap))


def _b0(ap_, reps, at=None):
    """Insert a broadcast dim of length `reps` into an AP."""
    lst = list(ap_.ap)
    pos = len(lst) if at is None else at
    lst.insert(pos, [0, reps])
    return bass.AP(tensor=ap_.tensor, offset=ap_.offset, ap=lst)


def _emit(ctx, tc, t, flags):
    nc = tc.nc
    has_bq, has_bk, has_bag, has_bab, has_btg = flags

    consts = ctx.enter_context(tc.tile_pool(name="consts", bufs=1))
    acts = ctx.enter_context(tc.tile_pool(name="acts", bufs=1))
    sb = ctx.enter_context(tc.tile_pool(name="sb", bufs=2))
    ps_tp = ctx.enter_context(tc.tile_pool(name="ps_tp", bufs=2, space="PSUM"))
    ps_mm = ctx.enter_context(tc.tile_pool(name="ps_mm", bufs=2, space="PSUM"))
    ps_pz = ctx.enter_context(tc.tile_pool(name="ps_pz", bufs=4, space="PSUM"))
    dramp = ctx.enter_context(tc.tile_pool(name="dram", bufs=1, space="DRAM"))
    dpp = ctx.enter_context(tc.tile_pool(name="dramP", bufs=16, space="DRAM"))

    # ---- constants / weights resident all kernel ----
    ident = consts.tile([128, 128], BF16)
    make_identity(nc, ident[:])
    eps_t = consts.tile([128, 1], F32)
    nc.vector.memset(eps_t[:], EPS)
    wbs_sb = consts.tile([CZ, 64], F8)
    svec_sb = consts.tile([128, H], F32)
    idx_sb = consts.tile([128, NT // 16], I16)
    bq_sb = bk_sb = None
    if has_bq:
        bq_sb = consts.tile([128, 4], F32)
        nc.sync.dma_start(bq_sb[:], t["bq"][:])
    if has_bk:
        bk_sb = consts.tile([128, 4], F32)
        nc.sync.dma_start(bk_sb[:], t["bk"][:])
    wq = consts.tile([128, 4, CS], BF16)
    wk = consts.tile([128, 4, CS], BF16)
    wv = consts.tile([128, 4, CS], BF16)
    wg = consts.tile([128, 4, CS], BF16)
    wout = consts.tile([128, 4, CS], BF16)

    # ---- persistent activations ----
    re = acts.tile([128, RT, CS], BF16)
    _re_src = t["re"].rearrange("(r p) c -> p r c", p=128)
    nc.sync.dma_start(wbs_sb[:], t["wbs"][:])
    nc.sync.dma_start(svec_sb[:], _bcast(t["svec"]))
    nc.sync.dma_start(idx_sb[:], t["idx"][:])
    for _r in range(RT):
        nc.sync.dma_start(re[:, _r, :], _re_src[:, _r, :])
    h_sb = acts.tile([128, RT, CS], BF16)
    xnT = acts.tile([128, 4, NT], BF16)
    qf = acts.tile([128, 4, NT], BF16)
    kf = acts.tile([128, 4, NT], BF16)
    qf2 = acts.tile([64, 4, NT], BF16)
    kf2 = acts.tile([64, 4, NT], BF16)
    gsig = acts.tile([128, RT, CS], BF16)
    bias_aa = acts.tile([128, RT, H, 64], BF16)
    w1 = acts.tile([128, 4, 2 * CS], BF16)
    w2 = acts.tile([128, 4, 2 * CS], BF16)
    wb = acts.tile([128, 8, CS], BF16)

    tbl = dramp.tile([NRES, 3 * CS], BF16)

    def ln_rstd(x_ap):
        """bn stats over free dim -> (mean [P,1], rstd [P,1]) tiles."""
        p = x_ap.shape[0]
        stats = sb.tile([128, 6], F32, tag="stats")
        nc.vector.bn_stats(stats[:p], x_ap)
        mv = sb.tile([128, 2], F32, tag="mv")
        nc.vector.bn_aggr(mv[:p], stats[:p])
        sd = sb.tile([128, 1], F32, tag="sd")
        nc.scalar.activation(sd[:p], mv[:p, 1:2], AF.Sqrt, bias=eps_t[:p], scale=1.0)
        nc.vector.reciprocal(sd[:p], sd[:p])
        return mv[:p, 0:1], sd[:p]

    from concourse.tile import add_dep_helper

    def attention(gp):
        # ---- v for both blocks first (mm slots free early) ----
        vts = []
        for g2 in range(2):
            g = 2 * gp + g2
            vp = ps_mm.tile([128, CS], F32, tag="mm")
            for k in range(4):
                nc.tensor.matmul(vp[0:64, :], xnT[:, k, g * 64:(g + 1) * 64],
                                 wv[:, k, :], start=(k == 0), stop=(k == 3))
            vt = sb.tile([64, CS], BF16, tag="vt", bufs=4)
            nc.vector.tensor_copy(vt[:], vp[0:64, :])
            vts.append(vt)
        # ---- attention: all heads, both blocks ----
        sc_ps = ps_pz.tile([128, CS], F32, tag="pz")
        for g2 in range(2):
            g = 2 * gp + g2
            for h in range(H):
                m = h // 2
                qsl = (qf[0:64, m, g * 64:(g + 1) * 64] if h % 2 == 0
                       else qf2[:, m, g * 64:(g + 1) * 64])
                ksl = (kf[0:64, m, g * 64:(g + 1) * 64] if h % 2 == 0
                       else kf2[:, m, g * 64:(g + 1) * 64])
                nc.tensor.matmul(sc_ps[g2 * 64:g2 * 64 + 64,
                                       h * 64:(h + 1) * 64],
                                 qsl, ksl, start=True, stop=True,
                                 tile_position=(0, g2 * 64))
        nc.vector.tensor_add(sc_ps[:].rearrange("p (h j) -> p h j", h=H),
                             sc_ps[:].rearrange("p (h j) -> p h j", h=H),
                             bias_aa[:, gp, :, :])
        a_sb = sb.tile([128, CS], BF16, tag="a_sb", bufs=3)
        nc.scalar.activation(a_sb[:], sc_ps[:], AF.Exp)
        rs = sb.tile([128, H], F32, tag="rs")
        nc.vector.tensor_reduce(rs[:], a_sb[:].rearrange(
            "p (h j) -> p h j", h=H), axis=mybir.AxisListType.X, op=ALU.add)
        rcp = sb.tile([128, H], F32, tag="rcp")
        nc.vector.reciprocal(rcp[:], rs[:])
        # fold softmax normalizer into the sigmoid gate
        gg = sb.tile([128, H, 64], BF16, tag="gg")
        nc.vector.tensor_mul(
            gg[:], gsig[:, gp, :].rearrange("p (h j) -> p h j", h=H),
            _b0(rcp[:], 64))

        o_ps = ps_pz.tile([128, CS], F32, tag="pz")
        for g2 in range(2):
            g = 2 * gp + g2
            vt = vts[g2]
            idq = ident[g2 * 64:g2 * 64 + 64, g2 * 64:g2 * 64 + 64]
            aT_ps = ps_tp.tile([64, CS], BF16, tag="tp")
            for h in range(H):
                nc.tensor.transpose(aT_ps[:, h * 64:(h + 1) * 64],
                                    a_sb[g2 * 64:g2 * 64 + 64,
                                         h * 64:(h + 1) * 64], idq)
            aT_sb = sb.tile([64, CS], BF16, tag="aT_sb", bufs=3)
            nc.vector.tensor_copy(aT_sb[:], aT_ps[:])
            for h in range(H):
                nc.tensor.matmul(
                    o_ps[g2 * 64:g2 * 64 + 64, h * 64:(h + 1) * 64],
                    aT_sb[:, h * 64:(h + 1) * 64],
                    vt[:, h * 64:(h + 1) * 64],
                    start=True, stop=True, tile_position=(0, g2 * 64))
        og_pair = sb.tile([128, CS], BF16, tag="og_pair", bufs=3)
        nc.vector.tensor_mul(og_pair[:].rearrange("p (h j) -> p h j", h=H),
                             o_ps[:].rearrange("p (h j) -> p h j", h=H),
                             gg[:])
        ogT = sb.tile([128, 4, 128], BF16, tag="ogT")
        for c in range(4):
            tp = ps_tp.tile([128, 128], BF16, tag="tp")
            nc.tensor.transpose(tp[:], og_pair[:, c * 128:(c + 1) * 128],
                                ident[:])
            nc.vector.tensor_copy(ogT[:, c, :], tp[:])
        # ---- Wout + residual (tp tag: freed late, off the mm path) ----
        pt = ps_tp.tile([128, CS], F32, tag="tp")
        for k in range(4):
            nc.tensor.matmul(pt[:], ogT[:, k, :], wout[:, k, :],
                             start=(k == 0), stop=(k == 3))
        nc.vector.tensor_add(h_sb[:, gp, :], pt[:], re[:, gp, :])

    # =============== phase A: LN1 + cond tables + bias path ===============
    with tc.tile_pool(name="pa", bufs=1) as pa, \
         tc.tile_pool(name="paw", bufs=2) as paw:
        wada = pa.tile([128, 3, 3 * CS], BF16)
        nc.scalar.dma_start(wada[:], t["wada"][:])
        bada_bc = pa.tile([128, 3 * CS], BF16)
        if has_bag or has_bab or has_btg:
            nc.sync.dma_start(bada_bc[:], _bcast(t["bada"]))

        dPs = {}

        def bias_block(g):
            """Pair-bias matmuls for block g -> dP(bf16) -> Pr_all[gp] half."""
            gp, g2 = g // 2, g % 2
            zt = paw.tile([CZ, 2, BLK * BLK], F8, tag="zt")
            zq = (nc.gpsimd, nc.scalar, nc.sync)[g % 3]
            zq.dma_start(zt[:], t["zz"][g])
            ze = ps_pz.tile([128, 512], F32, tag="pz")
            zo = ps_pz.tile([128, 512], F32, tag="pz")
            # z pass writes P rows 0..8 of each 32-strip; z^2 pass (host
            # precomputed) accumulates E[z^2] into row 9.  Each strip's
            # accumulation group closes before the next opens.
            for cg in range(4):
                tpos = (0, 32 * cg)
                rows = slice(32 * cg, 32 * cg + 32)
                ev = slice((2 * cg) * 512, (2 * cg + 1) * 512)
                od = slice((2 * cg + 1) * 512, (2 * cg + 2) * 512)
                nc.tensor.matmul(ze[rows, :], wbs_sb[:, 0:32], zt[:, 0, ev],
                                 start=True, stop=False, tile_position=tpos)
                nc.tensor.matmul(zo[rows, :], wbs_sb[:, 0:32], zt[:, 0, od],
                                 start=True, stop=False, tile_position=tpos)
                nc.tensor.matmul(ze[rows, :], wbs_sb[:, 32:64], zt[:, 1, ev],
                                 start=False, stop=True, tile_position=tpos)
                nc.tensor.matmul(zo[rows, :], wbs_sb[:, 32:64], zt[:, 1, od],
                                 start=False, stop=True, tile_position=tpos)
            Psbb = sb.tile([128, 1024], BF16, tag="Psbb")
            nc.vector.tensor_copy(Psbb[:, 0:512], ze[:])
            nc.vector.tensor_copy(Psbb[:, 512:1024], zo[:])
            # round-trip through DRAM to reshape [32cg+m, (ab i3 j)]
            # -> [i=(cg ab i3), m, j]; the strided re-load happens in
            # phase B so only dP (DRAM) holds the 16 blocks.
            dP = dpp.tile([128, 1024], BF16, tag="dP")
            st = nc.gpsimd.dma_start(dP[:], Psbb[:])
            dPs[g] = (dP, st.ins)

        def p1_tile(r):
            """Cond-table tile r: LN(s) @ [W_ada_gate|W_ada_bias|W_tgate]."""
            s_t = paw.tile([128, CC], BF16, tag="s_t")
            nc.sync.dma_start(s_t[:], t["s"][r * 128:(r + 1) * 128, :])
            mean, rstd = ln_rstd(s_t[:])
            cond = sb.tile([128, CC], BF16, tag="cond")
            nc.vector.tensor_scalar(out=cond[:], in0=s_t[:], scalar1=mean,
                                    scalar2=rstd, op0=ALU.subtract, op1=ALU.mult)
            ct = sb.tile([128, 3, 128], BF16, tag="ct")
            for c in range(3):
                tp = ps_tp.tile([128, 128], BF16, tag="tp")
                nc.tensor.transpose(tp[:], cond[:, c * 128:(c + 1) * 128], ident[:])
                nc.scalar.copy(ct[:, c, :], tp[:])
            tbl_sb = sb.tile([128, 3 * CS], BF16, tag="tbl_sb")
            for n in range(3):
                pt = ps_mm.tile([128, CS], F32, tag="mm")
                for k in range(3):
                    nc.tensor.matmul(pt[:], ct[:, k, :],
                                     wada[:, k, n * CS:(n + 1) * CS],
                                     start=(k == 0), stop=(k == 2))
                seg = slice(n * CS, (n + 1) * CS)
                if n == 0:
                    if has_bag:
                        nc.vector.tensor_add(pt[:], pt[:], bada_bc[:, seg])
                    nc.scalar.activation(tbl_sb[:, seg], pt[:], AF.Sigmoid)
                elif n == 1:
                    if has_bab:
                        nc.vector.tensor_add(tbl_sb[:, seg], pt[:], bada_bc[:, seg])
                    else:
                        nc.scalar.copy(tbl_sb[:, seg], pt[:])
                else:
                    if has_btg:
                        nc.vector.tensor_add(pt[:], pt[:], bada_bc[:, seg])
                    nc.scalar.activation(tbl_sb[:, seg], pt[:], AF.Sigmoid)
            nc.sync.dma_start(tbl[r * 128:(r + 1) * 128, :], tbl_sb[:])

        def ln1_tile(r):
            mean, rstd = ln_rstd(re[:, r, :])
            xn = sb.tile([128, CS], BF16, tag="xn")
            nc.vector.tensor_scalar(out=xn[:], in0=re[:, r, :], scalar1=mean,
                                    scalar2=rstd, op0=ALU.subtract, op1=ALU.mult)
            for c in range(4):
                tp = ps_tp.tile([128, 128], BF16, tag="tp")
                nc.tensor.transpose(tp[:], xn[:, c * 128:(c + 1) * 128], ident[:])
                nc.scalar.copy(xnT[:, c, r * 128:(r + 1) * 128], tp[:])

        def qk_proj(n):
            for (w, bias_sb, has_b, dst) in ((wq, bq_sb, has_bq, qf),
                                             (wk, bk_sb, has_bk, kf)):
                for m in range(4):
                    pt = ps_mm.tile([128, CS], F32, tag="mm")
                    for k in range(4):
                        nc.tensor.matmul(pt[:], w[:, k, m * 128:(m + 1) * 128],
                                         xnT[:, k, n * 512:(n + 1) * 512],
                                         start=(k == 0), stop=(k == 3))
                    dseg = dst[:, m, n * 512:(n + 1) * 512]
                    if has_b:
                        nc.vector.tensor_scalar_add(out=dseg, in0=pt[:],
                                                    scalar1=bias_sb[:, m:m + 1])
                    else:
                        nc.vector.tensor_copy(dseg, pt[:])
            # odd heads' q/k rows duplicated at partition base 0: every QK
            # matmul then issues from PE row-group 0 (mixed row-groups
            # draining into one PSUM bank concurrently crash the device)
            nsl = slice(n * 512, (n + 1) * 512)
            nc.sync.dma_start(qf2[:, :, nsl], qf[64:128, :, nsl])
            nc.sync.dma_start(kf2[:, :, nsl], kf[64:128, :, nsl])

        def g_proj(r):
            pt = ps_mm.tile([128, CS], F32, tag="mm")
            for k in range(4):
                nc.tensor.matmul(pt[:], xnT[:, k, r * 128:(r + 1) * 128],
                                 wg[:, k, :], start=(k == 0), stop=(k == 3))
            nc.scalar.activation(gsig[:, r, :], pt[:], AF.Sigmoid)

        def load_pr(gp):
            Pr = acts.tile([128, 10, 64], BF16, tag="Pr", bufs=2)
            for g2 in range(2):
                dP, st_ins = dPs[2 * gp + g2]
                base = dP[:]
                for cg in range(4):
                    srcap = bass.AP(tensor=base.tensor,
                                    offset=base.offset + cg * 32768,
                                    ap=[[64, 16], [1024, 10], [1, 64]])
                    q = (nc.sync, nc.gpsimd, nc.scalar)[(g2 * 4 + cg) % 3]
                    ld = q.dma_start(
                        Pr[g2 * 64 + cg * 16:g2 * 64 + (cg + 1) * 16, :, :],
                        srcap)
                    add_dep_helper(ld.ins, st_ins, reason="reshape RAW")
            return Pr

        def stats_prep(gp):
            """Pair-bias LN stats + full bias tile, off the critical chain."""
            Pr = load_pr(gp)
            msq = sb.tile([128, 64], F32, tag="msq")
            nc.vector.tensor_mul(msq[:], Pr[:, 8, :], Pr[:, 8, :])
            var_t = sb.tile([128, 64], F32, tag="var_t")
            nc.vector.tensor_sub(var_t[:], Pr[:, 9, :], msq[:])
            nc.scalar.activation(var_t[:], var_t[:], AF.Sqrt,
                                 bias=eps_t[:], scale=1.0)
            rstd_t = sb.tile([128, 64], F32, tag="rstd_t")
            nc.vector.reciprocal(rstd_t[:], var_t[:])
            mr_t = sb.tile([128, 64], F32, tag="mr_t")
            nc.vector.tensor_mul(mr_t[:], Pr[:, 8, :], rstd_t[:])
            # bias[p,(h,j)] = Pr_h*rstd - S_h*mean*rstd  (svec = -S)
            mrs = sb.tile([128, H, 64], BF16, tag="mrs")
            nc.vector.tensor_mul(mrs[:], _b0(mr_t[:], H, at=1),
                                 _b0(svec_sb[:], 64))
            ba = bias_aa[:, gp, :, :]
            nc.vector.tensor_mul(ba, Pr[:, 0:H, :], _b0(rstd_t[:], H, at=1))
            nc.vector.tensor_add(ba, ba, mrs[:])

        for u in range(RT):
            if u >= 1:
                stats_prep(u - 1)
            if u == 1:
                nc.sync.dma_start(wq[:], t["wq"][:])
                nc.sync.dma_start(wk[:], t["wk"][:])
                nc.sync.dma_start(wg[:], t["wg"][:])
            elif u == 2:
                nc.sync.dma_start(wv[:], t["wv"][:])
                nc.sync.dma_start(wout[:], t["wout"][:])
            elif u == 6:
                nc.gpsimd.dma_start(w1[:], t["w1"][:])
                nc.gpsimd.dma_start(w2[:], t["w2"][:])
                nc.gpsimd.dma_start(wb[:], t["wb"][:])
            ln1_tile(u)
            bias_block(2 * u)
            p1_tile(u)
            bias_block(2 * u + 1)
            if u >= 4:
                # fuse: attention for the first half overlaps the back half
                # of phase A (fills PE gaps, keeps HAM warm)
                attention(u - 4)
            if u == 3 or u == 7:
                n = u // 4
                qk_proj(n)
                for r in range(n * 4, n * 4 + 4):
                    g_proj(r)

        stats_prep(RT - 1)

    # =============== phase B/C: attention + transition ===============
    with tc.tile_pool(name="pb", bufs=1) as pb:
        tT = pb.tile([128, 4, NT], BF16)
        bb = pb.tile([128, 8, NT], BF16)
        tgate = pb.tile([128, RT, CS], BF16)

        def gather_r(r):
            gth_t = sb.tile([128, 1, 2 * CS], BF16, tag="gth")
            nc.gpsimd.dma_gather(
                out_ap=gth_t[:], in_ap=tbl[:, 0:2 * CS],
                idxs_ap=idx_sb[:, r * 8:(r + 1) * 8],
                num_idxs=128, num_idxs_reg=128, elem_size=2 * CS,
                elem_step=3 * CS)
            nc.gpsimd.dma_gather(
                out_ap=tgate[:, r:r + 1, :], in_ap=tbl[:, 2 * CS:3 * CS],
                idxs_ap=idx_sb[:, r * 8:(r + 1) * 8],
                num_idxs=128, num_idxs_reg=128, elem_size=CS,
                elem_step=3 * CS)
            return gth_t

        gths = {0: gather_r(0), 1: gather_r(1)}

        def ln2_tile(r, gth_t):
            mean, rstd = ln_rstd(h_sb[:, r, :])
            # t2 = LN(h)*sig(gate) + bias, fused as two scalar_tensor_tensor
            t1 = sb.tile([128, CS], BF16, tag="t1")
            nc.vector.scalar_tensor_tensor(
                out=t1[:], in0=h_sb[:, r, :], scalar=mean,
                in1=gth_t[:, 0, 0:CS], op0=ALU.subtract, op1=ALU.mult)
            t2 = sb.tile([128, CS], BF16, tag="t2")
            nc.vector.scalar_tensor_tensor(
                out=t2[:], in0=t1[:], scalar=rstd,
                in1=gth_t[:, 0, CS:2 * CS], op0=ALU.mult, op1=ALU.add)
            for c in range(4):
                tp = ps_tp.tile([128, 128], BF16, tag="tp")
                nc.tensor.transpose(tp[:], t2[:, c * 128:(c + 1) * 128], ident[:])
                nc.vector.tensor_copy(tT[:, c, r * 128:(r + 1) * 128], tp[:])

        def wb_r(r):
            pt = ps_mm.tile([128, CS], F32, tag="mm")
            for k in range(8):
                nc.tensor.matmul(pt[:], bb[:, k, r * 128:(r + 1) * 128],
                                 wb[:, k, :], start=(k == 0), stop=(k == 7))
            tr = sb.tile([128, CS], F32, tag="tr")
            nc.vector.tensor_mul(tr[:], pt[:], tgate[:, r, :])
            out_t = sb.tile([128, CS], F32, tag="out_t")
            nc.vector.tensor_add(out_t[:], tr[:], h_sb[:, r, :])
            nc.sync.dma_start(t["out"][r * 128:(r + 1) * 128, :], out_t[:])

        def w12(n):
            for m in range(8):
                p1 = ps_mm.tile([128, CS], F32, tag="mm")
                for k in range(4):
                    nc.tensor.matmul(p1[:], w1[:, k, m * 128:(m + 1) * 128],
                                     tT[:, k, n * 512:(n + 1) * 512],
                                     start=(k == 0), stop=(k == 3))
                u1s = sb.tile([128, 512], BF16, tag="u1s")
                nc.scalar.activation(u1s[:], p1[:], AF.Sigmoid)
                u1 = sb.tile([128, 512], BF16, tag="u1")
                nc.vector.tensor_mul(u1[:], u1s[:], p1[:])
                p2 = ps_pz.tile([128, CS], F32, tag="pz")
                for k in range(4):
                    nc.tensor.matmul(p2[:], w2[:, k, m * 128:(m + 1) * 128],
                                     tT[:, k, n * 512:(n + 1) * 512],
                                     start=(k == 0), stop=(k == 3))
                nc.vector.tensor_mul(bb[:, m, n * 512:(n + 1) * 512],
                                     u1[:], p2[:])

        for i in range(4):
            ln2_tile(i, gths.pop(i))
            gths[i + 2] = gather_r(i + 2)
            attention(4 + i)
        w12(0)
        for i in range(4, RT):
            ln2_tile(i, gths.pop(i))
            if i + 2 < RT:
                gths[i + 2] = gather_r(i + 2)
            wb_r(i - 4)
        w12(1)
        for r in range(4, RT):
            wb_r(r)


def build(flags):
    key = ("v2", flags)
    if key in _CACHE:
        return _CACHE[key]
    nc = bacc.Bacc("TRN2", target_bir_lowering=False, debug=False)
    t = _declare(nc)
    with tile.TileContext(nc) as tc:
        with ExitStack() as ctx:
            _emit(ctx, tc, t, flags)
    nc.compile()
    _CACHE[key] = nc
    return nc


def prep_core_inputs(inputs, core):
    """Host-side slicing + weight folding for one core."""
    b = core // 4
    g0 = (core % 4) * NBLK
    r0 = g0 * BLK

    f = lambda k: np.asarray(inputs[k], np.float32)
    ln_w, ln_b = f("ln_w"), f("ln_b")
    sc = 1.0 / np.sqrt(CH)

    def fold(w, scale=1.0):
        return ln_w[:, None] * np.asarray(w, np.float32) * scale

    def foldb(w, scale=1.0):
        return (ln_b @ np.asarray(w, np.float32)) * scale

    Wkv = f("Wkv")
    wq_h, bq_h = fold(inputs["Wq"], sc), foldb(inputs["Wq"], sc)
    wk_h, bk_h = fold(Wkv[:, :CS]), foldb(Wkv[:, :CS])
    wv_h, bv_h = fold(Wkv[:, CS:]), foldb(Wkv[:, CS:])
    wg_h, bg_h = fold(inputs["Wgate"]), foldb(inputs["Wgate"])
    if np.any(bv_h) or np.any(bg_h):
        raise NotImplementedError("nonzero folded v/gate bias unsupported")

    cw = f("adaln_cond_w")
    wada_h = np.concatenate(
        [cw[:, None] * f("W_ada_gate"), cw[:, None] * f("W_ada_bias"),
         cw[:, None] * f("W_tgate")], axis=1)
    bada_h = np.concatenate(
        [f("b_ada_gate"), np.zeros(CS, np.float32), f("b_tgate")]).astype(BF)

    wbias = f("bias_ln_w")[:, None] * f("Wbias")      # [128, 8]
    svec_h = (-wbias.sum(0)).astype(np.float32)       # minus S
    wbs_h = np.zeros((CZ, 64), np.float32)
    wbs_h[:, :H] = wbias
    wbs_h[:, 8] = 1.0 / CZ       # sum column directly produces the mean
    wbs_h[:, 32 + 9] = 1.0 / CZ  # z^2 pass accumulates E[z^2] into row 9

    def ktile(w, kt):
        w = np.asarray(w, np.float32)
        return np.ascontiguousarray(
            w.reshape(kt, 128, w.shape[1]).transpose(1, 0, 2)).astype(BF)

    # framepair: [16, 64, 64, 128] -> [16, 128, 4096] bf16
    fp = np.asarray(inputs["framepair_embed"][b, g0:g0 + NBLK], np.float32)
    zT = np.ascontiguousarray(
        fp.reshape(NBLK, BLK * BLK, CZ).transpose(0, 2, 1))
    zzs = np.stack([zT, zT * zT], axis=2).astype(F8NP)   # [NBLK, CZ, 2, 4096]

    idx = np.asarray(inputs["rigids_to_res_idx"][b, r0:r0 + NT]).astype(np.int16)
    idx_w = np.empty((128, NT // 16), np.int16)
    for p in range(16):
        idx_w[p] = idx[p::16]
    idx_w[16:] = np.tile(idx_w[:16], (7, 1))

    re_f32 = np.ascontiguousarray(inputs["rigids_embed"][b, r0:r0 + NT]).astype(np.float32)
    return {
        "re": re_f32.astype(BF),
        "zz": zzs,
        "s": np.ascontiguousarray(inputs["s"][b]).astype(BF),
        "idx": idx_w,
        "wq": ktile(wq_h, 4), "wk": ktile(wk_h, 4), "wv": ktile(wv_h, 4),
        "wg": ktile(wg_h, 4), "wout": ktile(inputs["Wout"], 4),
        "w1": ktile(inputs["W1"], 4), "w2": ktile(inputs["W2"], 4),
        "wb": ktile(inputs["Wb"], 8), "wada": ktile(wada_h, 3),
        "wbs": wbs_h.astype(F8NP), "svec": svec_h,
        "bq": np.ascontiguousarray(bq_h.reshape(4, 128).T),
        "bk": np.ascontiguousarray(bk_h.reshape(4, 128).T),
        "bada": bada_h,
    }, (bool(np.any(bq_h)), bool(np.any(bk_h)), bool(np.any(f("b_ada_gate"))),
        False, bool(np.any(f("b_tgate"))))


def kernel(**inputs):
    mask = np.asarray(inputs["rigids_mask"])
    if not np.all(mask == 1.0):
        print("WARNING: rigids_mask not all ones; kernel assumes ones", file=sys.stderr)

    in_maps, flags = [], None
    for core in range(NCORES):
        m, flags = prep_core_inputs(inputs, core)
        in_maps.append(m)

    nc = build(flags)
    res = run_bass_kernel_spmd(nc, in_maps, core_ids=list(range(NCORES)))

    out = np.empty((B, N, CS), np.float32)
    for core in range(NCORES):
        b = core // 4
        r0 = (core % 4) * NT
        out[b, r0:r0 + NT] = res.results[core]["out"]
    return out
